# revision 11
# baseline (speedup 1.0000x reference)
"""Net2DSeg fused gather + MLP kernel for 8 TRN2 cores.

Math (per sample b):
  feats_2d[p] = fmap[b, :, r_p, c_p]          (gather, c_p < 376 always)
  relu1      = relu(fusion @ W1 + b1)
  seg1       = feats_2d @ (W2[:64] @ Wc1) + relu1 @ (W2[64:] @ Wc1) + (b2 @ Wc1 + bc1)
  seg2       = feats_2d @ Wc2 + bc2

Device strategy (1 sample per core, chunk-major point order p = j*128 + i):
  - host pre-transposes the used map region to fmapT [141376, 64] rows
  - one indirect (SWDGE) gather pulls the 30080 needed rows into SBUF
  - fusionT ships fp16 [80, 30080] and stays fully resident in SBUF
  - per 512-pt block: one fp16 matmul -> r1 PSUM, one DVE relu -> L[64:]
  - per 128-pt chunk: PE-transpose gathered rows -> [64ch, 128pt] PSUM,
    DVE copy -> L[:64]; single matmul L-slice @ Wcomb [128, 38] yields
    both heads at once; DVE adds the fused bias.

Sync constraint: walrus allows only ONE semaphore wait per Matmult
(waits land on the fused LDWEIGHTS S3_LW struct).  Therefore:
  - float consts ship as ONE packed DMA; fusionT/w1 are single DMAs
  - identity comes from the host (no make_identity producer dep)
  - dummy PE ops absorb the const/w1 DMA-lane waits up front
  - every post-matmul op runs on DVE, and each PSUM tile is consumed by
    whole-tile DVE reads, so reuse + operand deps collapse into a
    single DVE wait per matmul.
"""

from contextlib import ExitStack

import numpy as np

import concourse.bass as bass
import concourse.tile as tile
from concourse import mybir
from concourse.bass_utils import run_bass_kernel_spmd

B = 8
C = 64
H = 376
WUSED = 376          # img_indices in [0, 376) for both coords
NPOS = H * WUSED     # 141376
NPTS = 30000
NPAD = 30080         # 235 * 128
DIN = 80
NCLS = 19
NOUT = 2 * NCLS      # 38
NCH = NPAD // 128    # 235 chunks of 128 points
CH_PER_BLK = 4       # mm1 block = 512 points
NBLK = 59            # 58 * 4 + 3 = 235
GATHER_PIECES = 5    # 47 chunks per piece

# const block column layout (f32, 128 partitions)
CB_IDENT = 0         # [128, 128]
CB_WC = 128          # [128, 38]
CB_BC = 166          # [128, 38]
CB_B1 = 204          # [64, 1]
CB_W = 205

F32 = mybir.dt.float32
F16 = mybir.dt.float16
I32 = mybir.dt.int32

_NC_CACHE = {}


def _emit(ctx: ExitStack, tc, fmapTd, fusTd, Fd, cbd, W1d, outd):
    nc = tc.nc
    const = ctx.enter_context(tc.tile_pool(name="const", bufs=1))
    gpool = ctx.enter_context(tc.tile_pool(name="gbig", bufs=1))
    spool = ctx.enter_context(tc.tile_pool(name="sbig", bufs=1))
    lpool = ctx.enter_context(tc.tile_pool(name="lhs", bufs=3))
    p1pool = ctx.enter_context(tc.tile_pool(name="psum1", bufs=2, space="PSUM"))
    p2pool = ctx.enter_context(tc.tile_pool(name="psum2", bufs=3, space="PSUM"))

    cb = const.tile([128, CB_W], F32)
    nc.sync.dma_start(cb[:], cbd[:])
    ft = const.tile([128, NCH], I32)
    nc.sync.dma_start(ft[:], Fd[:])
    w1h = const.tile([DIN, C], F16)
    nc.sync.dma_start(w1h[:], W1d[:])
    fuT = const.tile([DIN, NPAD], F16)
    nc.sync.dma_start(fuT[:], fusTd[:])

    ident = cb[:, CB_IDENT : CB_IDENT + 128]
    wc = cb[:, CB_WC : CB_WC + NOUT]
    bc = cb[:, CB_BC : CB_BC + NOUT]
    b1t = cb[0:C, CB_B1 : CB_B1 + 1]

    # warm-ups: absorb the cb / w1h DMA-lane waits on the PE and DVE
    # engines so no real compute instruction needs a second semaphore
    # wait for the constants (TPB instructions allow only ONE wait)
    dp1 = p2pool.tile([C, 128], F32, space="PSUM", tag="tp")
    nc.tensor.transpose(dp1[:], cb[:, 0:C], ident)
    dp2 = p2pool.tile([C, 128], F32, space="PSUM", tag="tp")
    nc.tensor.matmul(dp2[:, 0:C], lhsT=w1h[:], rhs=w1h[:], start=True, stop=True)
    dv = const.tile([128, 1], F32)
    nc.vector.tensor_copy(dv[:], cb[:, 0:1])

    G = gpool.tile([128, NCH, C], F32)       # gathered map rows, 7.7MB
    S = spool.tile([128, NCH, NOUT], F32)    # outputs, 4.6MB

    # [128,1] offset AP per chunk: 2D offset APs generate wrong descriptors
    # on real HW (sim accepts them), so one indirect DMA per 128-row chunk
    for jj in range(NCH):
        nc.gpsimd.indirect_dma_start(
            out=G[:, jj, :],
            out_offset=None,
            in_=fmapTd[:],
            in_offset=bass.IndirectOffsetOnAxis(ap=ft[:, jj : jj + 1], axis=0),
        )

    for blk in range(NBLK):
        nch = min(CH_PER_BLK, NCH - blk * CH_PER_BLK)
        npts = nch * 128
        r1 = p1pool.tile([C, CH_PER_BLK * 128], F32, space="PSUM")
        nc.tensor.matmul(
            r1[:, 0:npts],
            lhsT=w1h[:],
            rhs=fuT[:, blk * 512 : blk * 512 + npts],
            start=True,
            stop=True,
        )
        L = lpool.tile([128, CH_PER_BLK * 128], F32)
        nc.vector.tensor_scalar(
            L[C:128, 0:npts],
            r1[:, 0:npts],
            b1t,
            0.0,
            mybir.AluOpType.add,
            mybir.AluOpType.max,
        )
        for q in range(nch):
            jj = blk * CH_PER_BLK + q
            tp = p2pool.tile([C, 128], F32, space="PSUM")
            nc.tensor.transpose(tp[:], G[:, jj, :], ident)
            nc.vector.tensor_copy(L[0:C, q * 128 : (q + 1) * 128], tp[:])
            sp = p2pool.tile([128, NOUT], F32, space="PSUM")
            nc.tensor.matmul(
                sp[:],
                lhsT=L[:, q * 128 : (q + 1) * 128],
                rhs=wc,
                start=True,
                stop=True,
            )
            nc.vector.tensor_add(S[:, jj, :], sp[:], bc)

    # exactly 4 output DMAs: with the 4 input DMAs they fill DMAHW lanes
    # 0-7 with no wraparound, so no out DMA needs a lane-reuse wait
    # (PSEUDO_DMA_DIRECT2D also allows only ONE semaphore wait)
    oper = (NCH + 3) // 4
    for pc in range(4):
        j0, j1 = pc * oper, min((pc + 1) * oper, NCH)
        nc.sync.dma_start(outd[:, j0:j1, :], S[:, j0:j1, :])


def _legalize_waits(nc: bass.Bass) -> None:
    """walrus rejects any instruction with >1 semaphore wait.  Split the
    extras into single-wait NoOps on the same engine right before the
    instruction (same-queue program order preserves semantics)."""
    for fn in nc.m.functions:
        for bb in fn.blocks:
            insts = bb.instructions
            i = 0
            while i < len(insts):
                inst = insts[i]
                si = inst.sync_info
                if si is not None and len(si.on_wait) > 1:
                    waits = list(si.on_wait)
                    inst.sync_info = mybir.SyncInfo(
                        on_wait=waits[-1:], on_update=list(si.on_update)
                    )
                    for k, w in enumerate(waits[:-1]):
                        nop = mybir.InstNoOp(
                            name=f"{inst.name}-w{k}",
                            engine=inst.engine,
                            bass_nofuse=True,
                            sync_info=mybir.SyncInfo(on_wait=[w], on_update=[]),
                        )
                        nc.register_instruction(nop)
                        insts.insert(i + k, nop)
                    i += len(waits) - 1
                i += 1


def build_nc() -> bass.Bass:
    if "nc" in _NC_CACHE:
        return _NC_CACHE["nc"]
    nc = bass.Bass()
    fmapTd = nc.declare_dram_parameter("fmapT", [NPOS, C], F32, isOutput=False)
    fusTd = nc.declare_dram_parameter("fusT", [DIN, NPAD], F16, isOutput=False)
    Fd = nc.declare_dram_parameter("F", [128, NCH], I32, isOutput=False)
    cbd = nc.declare_dram_parameter("cblock", [128, CB_W], F32, isOutput=False)
    W1d = nc.declare_dram_parameter("W1", [DIN, C], F16, isOutput=False)
    outd = nc.declare_dram_parameter("out", [128, NCH, NOUT], F32, isOutput=True)
    with tile.TileContext(nc) as tc, ExitStack() as ctx:
        _emit(ctx, tc, fmapTd, fusTd, Fd, cbd, W1d, outd)
    _legalize_waits(nc)
    _NC_CACHE["nc"] = nc
    return nc


def prep_in_maps(inputs: dict) -> list[dict]:
    feats_full = np.asarray(inputs["feats_full"], np.float32)
    fusion = np.asarray(inputs["fusion_feats"], np.float32)
    idx = np.asarray(inputs["img_indices"], np.int32)
    W1 = np.asarray(inputs["W1"], np.float32)
    b1 = np.asarray(inputs["b1"], np.float32)
    W2 = np.asarray(inputs["W2"], np.float32)
    b2 = np.asarray(inputs["b2"], np.float32)
    Wc1 = np.asarray(inputs["Wc1"], np.float32)
    bc1 = np.asarray(inputs["bc1"], np.float32)
    Wc2 = np.asarray(inputs["Wc2"], np.float32)
    bc2 = np.asarray(inputs["bc2"], np.float32)

    A = W2[:C] @ Wc1
    G2 = W2[C:] @ Wc1
    c1 = b2 @ Wc1 + bc1

    cblock = np.zeros((128, CB_W), np.float32)
    cblock[:, CB_IDENT : CB_IDENT + 128] = np.eye(128, dtype=np.float32)
    cblock[:C, CB_WC : CB_WC + NCLS] = A
    cblock[:C, CB_WC + NCLS : CB_WC + NOUT] = Wc2
    cblock[C:, CB_WC : CB_WC + NCLS] = G2
    cblock[:, CB_BC : CB_BC + NOUT] = np.concatenate([c1, bc2])[None, :]
    cblock[:C, CB_B1] = b1
    W1h = np.ascontiguousarray(W1.astype(np.float16))

    in_maps = []
    for b in range(B):
        fmapT = np.ascontiguousarray(
            feats_full[b, :, :, :WUSED].transpose(1, 2, 0).reshape(NPOS, C)
        )
        fus = np.zeros((NPAD, DIN), np.float32)
        fus[:NPTS] = fusion[b * NPTS : (b + 1) * NPTS]
        fusT = np.ascontiguousarray(fus.T.astype(np.float16))
        Fflat = np.zeros(NPAD, np.int32)
        Fflat[:NPTS] = idx[b, :, 0] * WUSED + idx[b, :, 1]
        F_dev = np.ascontiguousarray(Fflat.reshape(NCH, 128).T)
        in_maps.append(
            dict(fmapT=fmapT, fusT=fusT, F=F_dev, cblock=cblock, W1=W1h)
        )
    return in_maps


def unshard(results: list[dict]) -> tuple[np.ndarray, np.ndarray]:
    seg1 = np.empty((B * NPTS, NCLS), np.float32)
    seg2 = np.empty((B * NPTS, NCLS), np.float32)
    for b in range(B):
        o = np.asarray(results[b]["out"]).reshape(128, NCH, NOUT)
        o2 = o.transpose(1, 0, 2).reshape(NPAD, NOUT)[:NPTS]
        seg1[b * NPTS : (b + 1) * NPTS] = o2[:, :NCLS]
        seg2[b * NPTS : (b + 1) * NPTS] = o2[:, NCLS:]
    return seg1, seg2


def kernel(**inputs):
    nc = build_nc()
    in_maps = prep_in_maps(inputs)
    res = run_bass_kernel_spmd(nc, in_maps, list(range(B)))
    return unshard(res.results)
